# revision 7
# baseline (speedup 1.0000x reference)
"""AttnBottleneck pooling kernel for Trainium2 (8 NeuronCores, data-parallel).

Computes, for x [B=128, S=512, D=5120]:
    scores = einsum('bsd,d->bs', x, attn_w)
    w      = softmax(scores, axis=-1)
    pooled = einsum('bsd,bs->bd', x, w)
    h      = relu(pooled @ w1 + b1)
    out    = h @ w2 + b2
Returns (out [B,17], w [B,512]).

Sharding: batch dim split 8 ways (16 batches per core), weights replicated.
x is read from HBM exactly once per core (memory-bound regime).
"""

import numpy as np

B, S, D = 128, 512, 5120
H1, H2 = 64, 17
N_CORES = 8
NB = B // N_CORES          # batches per core
P = 128                    # partitions
C = S // P                 # 4 s-chunks per batch
DC = D // P                # 40 d-chunks

_CACHE = {}


def _build_bass(nb=NB):
    import concourse.bass as bass
    import concourse.bacc as bacc
    import concourse.tile as tile
    from concourse import mybir

    f32 = mybir.dt.float32
    nc = bacc.Bacc(None, target_bir_lowering=False)

    x_d = nc.declare_dram_parameter("x", [nb, S, D], f32, isOutput=False)
    a_d = nc.declare_dram_parameter("a_vec", [D], f32, isOutput=False)
    w1_d = nc.declare_dram_parameter("w1r", [P, DC, H1], f32, isOutput=False)
    b1_d = nc.declare_dram_parameter("b1r", [1, H1], f32, isOutput=False)
    w2_d = nc.declare_dram_parameter("w2r", [H1, H2], f32, isOutput=False)
    b2_d = nc.declare_dram_parameter("b2r", [1, H2], f32, isOutput=False)
    out_d = nc.declare_dram_parameter("out", [nb, H2], f32, isOutput=True)
    wout_d = nc.declare_dram_parameter("w_out", [nb, S], f32, isOutput=True)

    ident_d = nc.inline_tensor(np.eye(P, dtype=np.float32), "ident")
    ones_col_d = nc.inline_tensor(np.ones((P, 1), dtype=np.float32), "ones_col")
    ones_row_d = nc.inline_tensor(np.ones((1, P), dtype=np.float32), "ones_row")

    with tile.TileContext(nc) as tc:
        with (
            tc.tile_pool(name="consts", bufs=1) as consts,
            tc.tile_pool(name="xp", bufs=2) as xp,
            tc.tile_pool(name="tmpp", bufs=1) as tmpp,
            tc.tile_pool(name="small", bufs=2) as small,
            tc.tile_pool(name="psum", bufs=1, space="PSUM") as psum,
        ):
            # ---- constants (loaded once) ----
            a_bc = consts.tile([P, D], f32)
            a_ap = a_d[:]
            a_bcast_src = bass.AP(
                tensor=a_ap.tensor, offset=a_ap.offset, ap=[[0, P], a_ap.ap[0]]
            )
            nc.gpsimd.dma_start(out=a_bc, in_=a_bcast_src)

            w1_sb = consts.tile([P, DC, H1], f32)
            nc.sync.dma_start(out=w1_sb, in_=w1_d[:])
            b1_sb = consts.tile([1, H1], f32)
            nc.sync.dma_start(out=b1_sb, in_=b1_d[:])
            w2_sb = consts.tile([H1, H2], f32)
            nc.sync.dma_start(out=w2_sb, in_=w2_d[:])
            b2_sb = consts.tile([1, H2], f32)
            nc.sync.dma_start(out=b2_sb, in_=b2_d[:])
            ident_sb = consts.tile([P, P], f32)
            nc.sync.dma_start(out=ident_sb, in_=ident_d[:])
            ones_col_sb = consts.tile([P, 1], f32)
            nc.sync.dma_start(out=ones_col_sb, in_=ones_col_d[:])
            ones_row_sb = consts.tile([1, P], f32)
            nc.sync.dma_start(out=ones_row_sb, in_=ones_row_d[:])
            one_sb = ones_row_sb[:, 0:1]

            # The fused multiply-reduce (scalar_tensor_tensor) cannot carry
            # semaphore waits in this compiler — all its dependencies must be
            # covered by DVE engine order. Touch a_bc once here; per-chunk
            # touches below cover the x DMA and the scores WAR hazard.
            touch = tmpp.tile([P, 1], f32, tag="touch")
            nc.vector.tensor_copy(touch, a_bc[:, 0:1])

            for b in range(nb):
                # ---- load x[b] as 4 chunks of [128 s, 5120 d] ----
                x_t = xp.tile([P, C, D], f32, tag="x")
                for c in range(C):
                    nc.sync.dma_start(
                        out=x_t[:, c, :], in_=x_d[b, c * P : (c + 1) * P, :]
                    )

                # ---- scores: fused multiply + free-dim reduce on DVE ----
                scores = small.tile([P, C], f32, tag="scores")
                # out is a throwaway (only accum_out matters): use a [P,1]
                # tile broadcast over the free dim so it is never materialized
                tmp = tmpp.tile([P, 1], f32, tag="tmp")
                for c in range(C):
                    # wait-carrier: reads the x chunk (DMA dep) and writes the
                    # scores slot (WAR dep), so the fused op below needs none
                    nc.vector.tensor_copy(
                        scores[:, c : c + 1], x_t[:, c, 0:1]
                    )
                    nc.vector.scalar_tensor_tensor(
                        out=tmp.broadcast_to((P, D)),
                        in0=x_t[:, c, :],
                        scalar=1.0,
                        in1=a_bc,
                        op0=mybir.AluOpType.mult,
                        op1=mybir.AluOpType.mult,
                        accum_out=scores[:, c : c + 1],
                    )

                # ---- softmax pieces (no max subtraction; scores ~ N(0,1)) ----
                E = small.tile([P, C], f32, tag="E")
                e_rs = small.tile([P, 1], f32, tag="e_rs")
                nc.scalar.activation(
                    out=E,
                    in_=scores,
                    func=mybir.ActivationFunctionType.Exp,
                    accum_out=e_rs,
                )

                # Z = sum over all 512 = cross-partition sum of e_rs (PE trick)
                z_p = psum.tile([1, 1], f32, tag="zsum")
                nc.tensor.matmul(z_p, e_rs, ones_col_sb, start=True, stop=True)
                z_sb = small.tile([1, 1], f32, tag="z_sb")
                nc.vector.tensor_copy(z_sb, z_p)
                zinv_sb = small.tile([1, 1], f32, tag="zinv")
                nc.vector.reciprocal(zinv_sb, z_p)

                # ---- w output: transpose E -> [4, 128], scale by 1/Z ----
                et_p = psum.tile([C, P], f32, tag="et")
                nc.tensor.transpose(et_p, E, ident_sb)
                z4_p = psum.tile([C, 1], f32, tag="z4")
                nc.tensor.matmul(
                    z4_p, ones_row_sb[:, 0:C], zinv_sb, start=True, stop=True
                )
                z4_sb = small.tile([C, 1], f32, tag="z4_sb")
                nc.vector.tensor_copy(z4_sb, z4_p)
                wt_sb = small.tile([C, P], f32, tag="wt")
                nc.scalar.activation(
                    out=wt_sb,
                    in_=et_p,
                    func=mybir.ActivationFunctionType.Copy,
                    scale=z4_sb,
                )
                nc.sync.dma_start(
                    out=wout_d[b].rearrange("(c p) -> c p", p=P), in_=wt_sb
                )

                # ---- pooling: pooledT[d] = sum_s E[s] * x[s, d]  (PE) ----
                pooled_p = psum.tile([P, DC], f32, tag="pooled")
                for dc in range(DC):
                    for c in range(C):
                        nc.tensor.matmul(
                            pooled_p[:, dc : dc + 1],
                            x_t[:, c, dc * P : (dc + 1) * P],
                            E[:, c : c + 1],
                            start=(c == 0),
                            stop=(c == C - 1),
                        )
                pooled_sb = small.tile([P, DC], f32, tag="pooled_sb")
                nc.scalar.activation(
                    out=pooled_sb,
                    in_=pooled_p,
                    func=mybir.ActivationFunctionType.Copy,
                )

                # ---- h = relu((pooledT.T @ w1)/Z + b1) ----
                h_p = psum.tile([1, H1], f32, tag="h")
                for dc in range(DC):
                    nc.tensor.matmul(
                        h_p,
                        pooled_sb[:, dc : dc + 1],
                        w1_sb[:, dc, :],
                        start=(dc == 0),
                        stop=False,
                    )
                # bias trick: add Z*b1 so the final 1/Z scale yields +b1
                nc.tensor.matmul(h_p, z_sb, b1_sb, start=False, stop=True)
                h_sb = small.tile([1, H1], f32, tag="h_sb")
                nc.scalar.activation(
                    out=h_sb,
                    in_=h_p,
                    func=mybir.ActivationFunctionType.Relu,
                    scale=zinv_sb,
                )

                # ---- out = h @ w2 + b2 ----
                ht_p = psum.tile([H1, 1], f32, tag="ht")
                nc.tensor.matmul(ht_p, h_sb, one_sb, start=True, stop=True)
                ht_sb = small.tile([H1, 1], f32, tag="ht_sb")
                nc.vector.tensor_copy(ht_sb, ht_p)
                o_p = psum.tile([1, H2], f32, tag="o")
                nc.tensor.matmul(o_p, ht_sb, w2_sb, start=True, stop=False)
                nc.tensor.matmul(o_p, one_sb, b2_sb, start=False, stop=True)
                o_sb = small.tile([1, H2], f32, tag="o_sb")
                nc.vector.tensor_copy(o_sb, o_p)
                nc.sync.dma_start(out=out_d[b : b + 1, :], in_=o_sb)

    nc.finalize()
    return nc


def _get_nc(nb=NB):
    if nb not in _CACHE:
        _CACHE[nb] = _build_bass(nb)
    return _CACHE[nb]


def _prep_maps(inputs, n_cores=N_CORES):
    x = np.ascontiguousarray(np.asarray(inputs["x"], dtype=np.float32))
    a = np.ascontiguousarray(np.asarray(inputs["attn_w"], dtype=np.float32))
    w1 = np.asarray(inputs["w1"], dtype=np.float32)
    b1 = np.asarray(inputs["b1"], dtype=np.float32)
    w2 = np.ascontiguousarray(np.asarray(inputs["w2"], dtype=np.float32))
    b2 = np.asarray(inputs["b2"], dtype=np.float32)

    # w1r[p, dc, j] = w1[dc*128 + p, j]
    w1r = np.ascontiguousarray(w1.reshape(DC, P, H1).transpose(1, 0, 2))
    b1r = np.ascontiguousarray(b1.reshape(1, H1))
    b2r = np.ascontiguousarray(b2.reshape(1, H2))

    nb = x.shape[0] // n_cores
    xs = x.reshape(n_cores, nb, S, D)
    in_maps = [
        {
            "x": xs[i],
            "a_vec": a,
            "w1r": w1r,
            "b1r": b1r,
            "w2r": w2,
            "b2r": b2r,
        }
        for i in range(n_cores)
    ]
    return in_maps, nb


def run(inputs, trace=False):
    """Run on hardware; returns (out, w, BassKernelResults)."""
    from concourse.bass_utils import run_bass_kernel_spmd

    in_maps, nb = _prep_maps(inputs)
    nc = _get_nc(nb)
    res = run_bass_kernel_spmd(nc, in_maps, list(range(N_CORES)), trace=trace)
    out = np.concatenate([r["out"] for r in res.results], axis=0)
    w = np.concatenate([r["w_out"] for r in res.results], axis=0)
    return out, w, res


def kernel(**inputs):
    out, w, _ = run(inputs, trace=False)
    return out, w


# revision 9
# speedup vs baseline: 2.2388x; 2.2388x over previous
"""AttnBottleneck pooling kernel for Trainium2 (8 NeuronCores, data-parallel).

Computes, for x [B=128, S=512, D=5120]:
    scores = einsum('bsd,d->bs', x, attn_w)
    w      = softmax(scores, axis=-1)
    pooled = einsum('bsd,bs->bd', x, w)
    h      = relu(pooled @ w1 + b1)
    out    = h @ w2 + b2
Returns (out [B,17], w [B,512]).

Sharding: batch dim split 8 ways (16 batches per core), weights replicated.
x is read from HBM exactly once per core (memory-bound regime).

Structure per core:
  per batch b (pipelined; DMA of b+1 overlaps compute of b):
    - DMA x[b] into SBUF as 4 chunks [128s, 5120d]
    - scores via fused multiply+reduce on DVE (scalar_tensor_tensor)
    - softmax pieces on ACT/PE (exp with accumulated row-sums; cross-
      partition sum via a ones-matmul; no max subtraction - scores ~ N(0,1))
    - w output: PE-transpose of E, scaled by 1/Z
    - pooling on PE: 40 matmuls with E-column stationary, x moving (N=512),
      giving unnormalized pooled [1, 5120] in PSUM; staged to a DRAM scratch
  epilogue (once): gather pooled for all 16 batches as [16, 5120], PE-block-
    transpose to [128, 40, 16], then the tiny 2-layer head for all batches
    at once (bias + 1/Z normalization folded in via rank-1 matmul tricks).
"""

import numpy as np

B, S, D = 128, 512, 5120
H1, H2 = 64, 17
N_CORES = 8
NB = B // N_CORES          # batches per core
P = 128                    # partitions
C = S // P                 # 4 s-chunks per batch
DC = D // P                # 40 d-chunks
NCH = 10                   # pooling free-dim chunks of 512
HALF = D // 2              # pooled is staged to DRAM in two 2560 halves

_CACHE = {}


def _build_bass(nb=NB):
    import concourse.bass as bass
    import concourse.bacc as bacc
    import concourse.tile as tile
    from concourse import mybir

    f32 = mybir.dt.float32
    nc = bacc.Bacc(None, target_bir_lowering=False)

    x_d = nc.declare_dram_parameter("x", [nb, S, D], f32, isOutput=False)
    a_d = nc.declare_dram_parameter("a_vec", [D], f32, isOutput=False)
    w1_d = nc.declare_dram_parameter("w1r", [P, DC, H1], f32, isOutput=False)
    b1_d = nc.declare_dram_parameter("b1r", [1, H1], f32, isOutput=False)
    w2_d = nc.declare_dram_parameter("w2r", [H1, H2], f32, isOutput=False)
    b2_d = nc.declare_dram_parameter("b2r", [1, H2], f32, isOutput=False)
    out_d = nc.declare_dram_parameter("out", [nb, H2], f32, isOutput=True)
    wout_d = nc.declare_dram_parameter("w_out", [nb, S], f32, isOutput=True)

    ident_d = nc.inline_tensor(np.eye(P, dtype=np.float32), "ident")
    ones_col_d = nc.inline_tensor(np.ones((P, 1), dtype=np.float32), "ones_col")
    ones_row_d = nc.inline_tensor(np.ones((1, P), dtype=np.float32), "ones_row")

    pooled_scratch = nc.dram_tensor("pooled_scratch", [nb, D], f32)

    with tile.TileContext(nc) as tc:
        with (
            tc.tile_pool(name="consts", bufs=1) as consts,
            tc.tile_pool(name="xp", bufs=2) as xp,
            tc.tile_pool(name="stage", bufs=1) as stagep,
            tc.tile_pool(name="small", bufs=2) as small,
            tc.tile_pool(name="psum", bufs=1, space="PSUM") as psum,
        ):
            # ---- constants (loaded once) ----
            a_bc = consts.tile([P, D], f32)
            a_ap = a_d[:]
            a_bcast_src = bass.AP(
                tensor=a_ap.tensor, offset=a_ap.offset, ap=[[0, P], a_ap.ap[0]]
            )
            nc.gpsimd.dma_start(out=a_bc, in_=a_bcast_src)

            w1_sb = consts.tile([P, DC, H1], f32)
            nc.sync.dma_start(out=w1_sb, in_=w1_d[:])
            b1_sb = consts.tile([1, H1], f32)
            nc.sync.dma_start(out=b1_sb, in_=b1_d[:])
            w2_sb = consts.tile([H1, H2], f32)
            nc.sync.dma_start(out=w2_sb, in_=w2_d[:])
            b2_sb = consts.tile([1, H2], f32)
            nc.sync.dma_start(out=b2_sb, in_=b2_d[:])
            ident_sb = consts.tile([P, P], f32)
            nc.sync.dma_start(out=ident_sb, in_=ident_d[:])
            ones_col_sb = consts.tile([P, 1], f32)
            nc.sync.dma_start(out=ones_col_sb, in_=ones_col_d[:])
            ones_row_sb = consts.tile([1, P], f32)
            nc.sync.dma_start(out=ones_row_sb, in_=ones_row_d[:])

            # per-batch softmax denominators, kept on partition 0
            z_keep = consts.tile([1, nb], f32)

            # The fused multiply-reduce (scalar_tensor_tensor) cannot carry
            # semaphore waits in this compiler - all its dependencies must be
            # covered by DVE engine order. Touch a_bc once here; per-chunk
            # touches below cover the x DMA and the scores WAR hazard.
            touch = consts.tile([P, 1], f32)
            nc.vector.tensor_copy(touch, a_bc[:, 0:1])

            for b in range(nb):
                # ---- load x[b] as 4 chunks of [128 s, 5120 d] ----
                x_t = xp.tile([P, C, D], f32, tag="x")
                for c in range(C):
                    nc.sync.dma_start(
                        out=x_t[:, c, :], in_=x_d[b, c * P : (c + 1) * P, :]
                    )

                # ---- scores: fused multiply + free-dim reduce on DVE ----
                scores = small.tile([P, C], f32, tag="scores")
                tmp = small.tile([P, 1], f32, tag="tmp")
                for c in range(C):
                    # wait-carrier: reads the x chunk (DMA dep) and writes the
                    # scores slot (WAR dep), so the fused op below needs none
                    nc.vector.tensor_copy(scores[:, c : c + 1], x_t[:, c, 0:1])
                    nc.vector.scalar_tensor_tensor(
                        out=tmp.broadcast_to((P, D)),
                        in0=x_t[:, c, :],
                        scalar=1.0,
                        in1=a_bc,
                        op0=mybir.AluOpType.mult,
                        op1=mybir.AluOpType.mult,
                        accum_out=scores[:, c : c + 1],
                    )

                # ---- softmax pieces (no max subtraction; scores ~ N(0,1)) ----
                E = small.tile([P, C], f32, tag="E")
                e_rs = small.tile([P, 1], f32, tag="e_rs")
                nc.scalar.activation(
                    out=E,
                    in_=scores,
                    func=mybir.ActivationFunctionType.Exp,
                    accum_out=e_rs,
                )

                # Z = sum over all 512 = cross-partition sum of e_rs (PE trick)
                z_p = psum.tile([1, 1], f32, tag="z")
                nc.tensor.matmul(z_p, e_rs, ones_col_sb, start=True, stop=True)
                nc.vector.tensor_copy(z_keep[:, b : b + 1], z_p)
                zinv_sb = small.tile([1, 1], f32, tag="zinv")
                nc.vector.reciprocal(zinv_sb, z_p)

                # ---- w output: transpose E -> [4, 128], scale by 1/Z ----
                et_p = psum.tile([C, P], f32, tag="et")
                nc.tensor.transpose(et_p, E, ident_sb)
                z4_p = psum.tile([C, 1], f32, tag="z4")
                nc.tensor.matmul(
                    z4_p, ones_row_sb[:, 0:C], zinv_sb, start=True, stop=True
                )
                z4_sb = small.tile([C, 1], f32, tag="z4_sb")
                nc.vector.tensor_copy(z4_sb, z4_p)
                wt_sb = small.tile([C, P], f32, tag="wt")
                nc.scalar.activation(
                    out=wt_sb,
                    in_=et_p,
                    func=mybir.ActivationFunctionType.Copy,
                    scale=z4_sb,
                )
                nc.sync.dma_start(
                    out=wout_d[b].rearrange("(c p) -> c p", p=P), in_=wt_sb
                )

                # ---- pooling: pooled[d] = sum_s E[s] * x[s, d]  (PE) ----
                # E-column stationary, x moving (N=512): pooled lands as
                # [1, 2560] PSUM halves, staged through SBUF to DRAM scratch.
                for half in range(2):
                    pool_p = psum.tile([1, HALF], f32, tag="pool")
                    for nch in range(NCH // 2):
                        off = nch * 512
                        xoff = half * HALF + off
                        for c in range(C):
                            nc.tensor.matmul(
                                pool_p[:, off : off + 512],
                                E[:, c : c + 1],
                                x_t[:, c, xoff : xoff + 512],
                                start=(c == 0),
                                stop=(c == C - 1),
                            )
                    stage_sb = stagep.tile([1, HALF], f32, tag="stage")
                    nc.scalar.activation(
                        out=stage_sb,
                        in_=pool_p,
                        func=mybir.ActivationFunctionType.Copy,
                    )
                    nc.gpsimd.dma_start(
                        out=pooled_scratch[b, half * HALF : (half + 1) * HALF],
                        in_=stage_sb,
                    )

            # ---- epilogue: 2-layer head for all nb batches at once ----
            pooled_all = xp.tile([nb, D], f32, tag="x")
            nc.sync.dma_start(out=pooled_all, in_=pooled_scratch[:])

            # per-batch 1/Z column [nb, 1]: move z_keep across partitions
            z_col = small.tile([nb, 1], f32, tag="z_col")
            nc.sync.dma_start(
                out=z_col, in_=z_keep
            )
            zinv_col = small.tile([nb, 1], f32, tag="zinv_col")
            nc.vector.reciprocal(zinv_col, z_col)

            # block-transpose pooled_all into [128, 40, nb]
            poolt_p = psum.tile([P, DC, nb], f32, tag="pool")
            for dc in range(DC):
                nc.tensor.transpose(
                    poolt_p[:, dc, :],
                    pooled_all[:, dc * P : (dc + 1) * P],
                    ident_sb[0:nb, 0:nb],
                )
            poolt_sb = xp.tile([P, DC, nb], f32, tag="x")
            nc.scalar.activation(
                out=poolt_sb,
                in_=poolt_p,
                func=mybir.ActivationFunctionType.Copy,
            )

            # h = relu((pooledT.T @ w1)/Z + b1)   [nb, 64]
            h_p = psum.tile([nb, H1], f32, tag="et")
            for dc in range(DC):
                nc.tensor.matmul(
                    h_p,
                    poolt_sb[:, dc, :],
                    w1_sb[:, dc, :],
                    start=(dc == 0),
                    stop=False,
                )
            # bias trick: add Z_b*b1 so the final 1/Z scale yields +b1
            nc.tensor.matmul(h_p, z_keep, b1_sb, start=False, stop=True)
            h_sb = small.tile([nb, H1], f32, tag="h_sb")
            nc.scalar.activation(
                out=h_sb,
                in_=h_p,
                func=mybir.ActivationFunctionType.Relu,
                scale=zinv_col,
            )

            # out = h @ w2 + b2   [nb, 17]
            ht_p = psum.tile([H1, nb], f32, tag="z4")
            nc.tensor.transpose(ht_p, h_sb, ident_sb[0:nb, 0:nb])
            ht_sb = small.tile([H1, nb], f32, tag="ht_sb")
            nc.vector.tensor_copy(ht_sb, ht_p)
            o_p = psum.tile([nb, H2], f32, tag="z")
            nc.tensor.matmul(o_p, ht_sb, w2_sb, start=True, stop=False)
            nc.tensor.matmul(
                o_p, ones_row_sb[:, 0:nb], b2_sb, start=False, stop=True
            )
            o_sb = small.tile([nb, H2], f32, tag="o_sb")
            nc.vector.tensor_copy(o_sb, o_p)
            nc.sync.dma_start(out=out_d[:], in_=o_sb)

    nc.finalize()
    return nc


def _get_nc(nb=NB):
    if nb not in _CACHE:
        _CACHE[nb] = _build_bass(nb)
    return _CACHE[nb]


def _prep_maps(inputs, n_cores=N_CORES):
    x = np.ascontiguousarray(np.asarray(inputs["x"], dtype=np.float32))
    a = np.ascontiguousarray(np.asarray(inputs["attn_w"], dtype=np.float32))
    w1 = np.asarray(inputs["w1"], dtype=np.float32)
    b1 = np.asarray(inputs["b1"], dtype=np.float32)
    w2 = np.ascontiguousarray(np.asarray(inputs["w2"], dtype=np.float32))
    b2 = np.asarray(inputs["b2"], dtype=np.float32)

    # w1r[p, dc, j] = w1[dc*128 + p, j]
    w1r = np.ascontiguousarray(w1.reshape(DC, P, H1).transpose(1, 0, 2))
    b1r = np.ascontiguousarray(b1.reshape(1, H1))
    b2r = np.ascontiguousarray(b2.reshape(1, H2))

    nb = x.shape[0] // n_cores
    xs = x.reshape(n_cores, nb, S, D)
    in_maps = [
        {
            "x": xs[i],
            "a_vec": a,
            "w1r": w1r,
            "b1r": b1r,
            "w2r": w2,
            "b2r": b2r,
        }
        for i in range(n_cores)
    ]
    return in_maps, nb


def run(inputs, trace=False):
    """Run on hardware; returns (out, w, BassKernelResults)."""
    from concourse.bass_utils import run_bass_kernel_spmd

    in_maps, nb = _prep_maps(inputs)
    nc = _get_nc(nb)
    res = run_bass_kernel_spmd(nc, in_maps, list(range(N_CORES)), trace=trace)
    out = np.concatenate([r["out"] for r in res.results], axis=0)
    w = np.concatenate([r["w_out"] for r in res.results], axis=0)
    return out, w, res


def kernel(**inputs):
    out, w, _ = run(inputs, trace=False)
    return out, w


# revision 12
# speedup vs baseline: 3.1025x; 1.3858x over previous
"""AttnBottleneck pooling kernel for Trainium2 (8 NeuronCores, data-parallel).

Computes, for x [B=128, S=512, D=5120]:
    scores = einsum('bsd,d->bs', x, attn_w)
    w      = softmax(scores, axis=-1)
    pooled = einsum('bsd,bs->bd', x, w)
    h      = relu(pooled @ w1 + b1)
    out    = h @ w2 + b2
Returns (out [B,17], w [B,512]).

Sharding: batch dim split 8 ways (16 batches per core), weights replicated.
x is read from HBM exactly once per core (memory-bound regime).

Structure per core:
  per batch b (pipelined; DMA of b+1 overlaps compute of b):
    - DMA x[b] into SBUF as 4 chunks [128s, 5120d]
    - scores via fused multiply+reduce on DVE (scalar_tensor_tensor)
    - softmax pieces on ACT/PE (exp with accumulated row-sums; cross-
      partition sum via a ones-matmul; no max subtraction - scores ~ N(0,1))
    - w output: PE-transpose of E, scaled by 1/Z
    - pooling on PE: 40 matmuls with E-column stationary, x moving (N=512),
      giving unnormalized pooled [1, 5120] in PSUM; staged to a DRAM scratch
  epilogue (once): gather pooled for all 16 batches as [16, 5120], PE-block-
    transpose to [128, 40, 16], then the tiny 2-layer head for all batches
    at once (bias + 1/Z normalization folded in via rank-1 matmul tricks).
"""

import numpy as np

B, S, D = 128, 512, 5120
H1, H2 = 64, 17
N_CORES = 8
NB = B // N_CORES          # batches per core
P = 128                    # partitions
C = S // P                 # 4 s-chunks per batch
DC = D // P                # 40 d-chunks
NCH = 10                   # pooling free-dim chunks of 512
HALF = D // 2              # pooled is staged to DRAM in two 2560 halves

_CACHE = {}


def _build_bass(nb=NB):
    import concourse.bass as bass
    import concourse.bacc as bacc
    import concourse.tile as tile
    from concourse import mybir

    f32 = mybir.dt.float32
    f32r = mybir.dt.float32r
    nc = bacc.Bacc(None, target_bir_lowering=False)

    x_d = nc.declare_dram_parameter("x", [nb, S, D], f32, isOutput=False)
    a_d = nc.declare_dram_parameter("a_vec", [D], f32, isOutput=False)
    w1_d = nc.declare_dram_parameter("w1r", [P, DC, H1], f32, isOutput=False)
    b1_d = nc.declare_dram_parameter("b1r", [1, H1], f32, isOutput=False)
    w2_d = nc.declare_dram_parameter("w2r", [H1, H2], f32, isOutput=False)
    b2_d = nc.declare_dram_parameter("b2r", [1, H2], f32, isOutput=False)
    out_d = nc.declare_dram_parameter("out", [nb, H2], f32, isOutput=True)
    wout_d = nc.declare_dram_parameter("w_out", [nb, S], f32, isOutput=True)

    ident_d = nc.inline_tensor(np.eye(P, dtype=np.float32), "ident")
    ones_col_d = nc.inline_tensor(np.ones((P, 1), dtype=np.float32), "ones_col")
    ones_row_d = nc.inline_tensor(np.ones((1, P), dtype=np.float32), "ones_row")

    pooled_scratch = nc.dram_tensor("pooled_scratch", [nb, D], f32)

    with tile.TileContext(nc) as tc:
        with (
            tc.tile_pool(name="consts", bufs=1) as consts,
            tc.tile_pool(name="xp", bufs=2) as xp,
            tc.tile_pool(name="stage", bufs=1) as stagep,
            tc.tile_pool(name="small", bufs=2) as small,
            tc.tile_pool(name="psum", bufs=1, space="PSUM") as psum,
        ):
            # ---- constants (loaded once) ----
            a_bc = consts.tile([P, D], f32)
            a_ap = a_d[:]
            a_bcast_src = bass.AP(
                tensor=a_ap.tensor, offset=a_ap.offset, ap=[[0, P], a_ap.ap[0]]
            )
            nc.gpsimd.dma_start(out=a_bc, in_=a_bcast_src)

            w1_sb = consts.tile([P, DC, H1], f32r)
            nc.sync.dma_start(out=w1_sb, in_=w1_d[:].bitcast(f32r))
            b1_sb = consts.tile([1, H1], f32)
            nc.sync.dma_start(out=b1_sb, in_=b1_d[:])
            w2_sb = consts.tile([H1, H2], f32)
            nc.sync.dma_start(out=w2_sb, in_=w2_d[:])
            b2_sb = consts.tile([1, H2], f32)
            nc.sync.dma_start(out=b2_sb, in_=b2_d[:])
            ident_sb = consts.tile([P, P], f32)
            nc.sync.dma_start(out=ident_sb, in_=ident_d[:])
            ones_col_sb = consts.tile([P, 1], f32)
            nc.sync.dma_start(out=ones_col_sb, in_=ones_col_d[:])
            ones_row_sb = consts.tile([1, P], f32)
            nc.sync.dma_start(out=ones_row_sb, in_=ones_row_d[:])

            # per-batch softmax denominators, kept on partition 0
            z_keep = consts.tile([1, nb], f32)

            # The fused multiply-reduce (scalar_tensor_tensor) cannot carry
            # semaphore waits in this compiler - all its dependencies must be
            # covered by DVE engine order. Touch a_bc once here; per-chunk
            # touches below cover the x DMA and the scores WAR hazard.
            touch = consts.tile([P, 1], f32)
            nc.vector.tensor_copy(touch, a_bc[:, 0:1])

            for b in range(nb):
                # ---- load x[b] as 4 chunks of [128 s, 5120 d] ----
                x_t = xp.tile([P, C, D], f32r, tag="x")
                for c in range(C):
                    nc.sync.dma_start(
                        out=x_t[:, c, :],
                        in_=x_d[b, c * P : (c + 1) * P, :].bitcast(f32r),
                    )

                # ---- scores: fused multiply + free-dim reduce on DVE ----
                scores = small.tile([P, C], f32, tag="scores")
                tmp = small.tile([P, 1], f32, tag="tmp")
                for c in range(C):
                    # wait-carrier: reads the x chunk (DMA dep) and writes the
                    # scores slot (WAR dep), so the fused op below needs none
                    nc.vector.tensor_copy(scores[:, c : c + 1], x_t[:, c, 0:1].bitcast(f32))
                    nc.vector.scalar_tensor_tensor(
                        out=tmp.broadcast_to((P, D)),
                        in0=x_t[:, c, :].bitcast(f32),
                        scalar=1.0,
                        in1=a_bc,
                        op0=mybir.AluOpType.mult,
                        op1=mybir.AluOpType.mult,
                        accum_out=scores[:, c : c + 1],
                    )

                # ---- softmax pieces (no max subtraction; scores ~ N(0,1)) ----
                E = small.tile([P, C], f32, tag="E")
                e_rs = small.tile([P, 1], f32, tag="e_rs")
                nc.scalar.activation(
                    out=E,
                    in_=scores,
                    func=mybir.ActivationFunctionType.Exp,
                    accum_out=e_rs,
                )

                # Z = sum over all 512 = cross-partition sum of e_rs (PE trick)
                z_p = psum.tile([1, 1], f32, tag="z")
                nc.tensor.matmul(z_p, e_rs, ones_col_sb, start=True, stop=True)
                nc.vector.tensor_copy(z_keep[:, b : b + 1], z_p)
                zinv_sb = small.tile([1, 1], f32, tag="zinv")
                nc.vector.reciprocal(zinv_sb, z_p)

                # ---- w output: transpose E -> [4, 128], scale by 1/Z ----
                et_p = psum.tile([C, P], f32, tag="et")
                nc.tensor.transpose(et_p, E, ident_sb)
                z4_p = psum.tile([C, 1], f32, tag="z4")
                nc.tensor.matmul(
                    z4_p, ones_row_sb[:, 0:C], zinv_sb, start=True, stop=True
                )
                z4_sb = small.tile([C, 1], f32, tag="z4_sb")
                nc.vector.tensor_copy(z4_sb, z4_p)
                wt_sb = small.tile([C, P], f32, tag="wt")
                nc.scalar.activation(
                    out=wt_sb,
                    in_=et_p,
                    func=mybir.ActivationFunctionType.Copy,
                    scale=z4_sb,
                )
                nc.sync.dma_start(
                    out=wout_d[b].rearrange("(c p) -> c p", p=P), in_=wt_sb
                )

                e_r = small.tile([P, C], f32r, tag="e_r")
                nc.vector.tensor_copy(e_r, E)

                # ---- pooling: pooled[d] = sum_s E[s] * x[s, d]  (PE) ----
                # E-column stationary, x moving (N=512): pooled lands as
                # [1, 2560] PSUM halves, staged through SBUF to DRAM scratch.
                for half in range(2):
                    pool_p = psum.tile([1, HALF], f32, tag="pool")
                    for nch in range(NCH // 2):
                        off = nch * 512
                        xoff = half * HALF + off
                        for c in range(C):
                            nc.tensor.matmul(
                                pool_p[:, off : off + 512],
                                e_r[:, c : c + 1],
                                x_t[:, c, xoff : xoff + 512],
                                start=(c == 0),
                                stop=(c == C - 1),
                            )
                    stage_sb = stagep.tile([1, HALF], f32, tag="stage")
                    nc.scalar.activation(
                        out=stage_sb,
                        in_=pool_p,
                        func=mybir.ActivationFunctionType.Copy,
                    )
                    nc.gpsimd.dma_start(
                        out=pooled_scratch[b, half * HALF : (half + 1) * HALF],
                        in_=stage_sb,
                    )

            # ---- epilogue: 2-layer head for all nb batches at once ----
            pooled_all = xp.tile([nb, D], f32, tag="x")
            nc.sync.dma_start(out=pooled_all, in_=pooled_scratch[:])

            # per-batch 1/Z column [nb, 1]: move z_keep across partitions
            z_col = small.tile([nb, 1], f32, tag="z_col")
            nc.sync.dma_start(
                out=z_col, in_=z_keep
            )
            zinv_col = small.tile([nb, 1], f32, tag="zinv_col")
            nc.vector.reciprocal(zinv_col, z_col)

            # block-transpose pooled_all into [128, 40, nb]
            poolt_p = psum.tile([P, DC, nb], f32, tag="pool")
            for dc in range(DC):
                nc.tensor.transpose(
                    poolt_p[:, dc, :],
                    pooled_all[:, dc * P : (dc + 1) * P],
                    ident_sb[0:nb, 0:nb],
                )
            poolt_sb = xp.tile([P, DC, nb], f32r, tag="x")
            nc.scalar.activation(
                out=poolt_sb,
                in_=poolt_p,
                func=mybir.ActivationFunctionType.Copy,
            )

            # h = relu((pooledT.T @ w1)/Z + b1)   [nb, 64]
            h_p = psum.tile([nb, H1], f32, tag="et")
            for dc in range(DC):
                nc.tensor.matmul(
                    h_p,
                    poolt_sb[:, dc, :],
                    w1_sb[:, dc, :],
                    start=(dc == 0),
                    stop=False,
                )
            # bias trick: add Z_b*b1 so the final 1/Z scale yields +b1
            nc.tensor.matmul(h_p, z_keep, b1_sb, start=False, stop=True)
            h_sb = small.tile([nb, H1], f32, tag="h_sb")
            nc.scalar.activation(
                out=h_sb,
                in_=h_p,
                func=mybir.ActivationFunctionType.Relu,
                scale=zinv_col,
            )

            # out = h @ w2 + b2   [nb, 17]
            ht_p = psum.tile([H1, nb], f32, tag="z4")
            nc.tensor.transpose(ht_p, h_sb, ident_sb[0:nb, 0:nb])
            ht_sb = small.tile([H1, nb], f32, tag="ht_sb")
            nc.vector.tensor_copy(ht_sb, ht_p)
            o_p = psum.tile([nb, H2], f32, tag="z")
            nc.tensor.matmul(o_p, ht_sb, w2_sb, start=True, stop=False)
            nc.tensor.matmul(
                o_p, ones_row_sb[:, 0:nb], b2_sb, start=False, stop=True
            )
            o_sb = small.tile([nb, H2], f32, tag="o_sb")
            nc.vector.tensor_copy(o_sb, o_p)
            nc.sync.dma_start(out=out_d[:], in_=o_sb)

    nc.finalize()
    return nc


def _get_nc(nb=NB):
    if nb not in _CACHE:
        _CACHE[nb] = _build_bass(nb)
    return _CACHE[nb]


def _prep_maps(inputs, n_cores=N_CORES):
    x = np.ascontiguousarray(np.asarray(inputs["x"], dtype=np.float32))
    a = np.ascontiguousarray(np.asarray(inputs["attn_w"], dtype=np.float32))
    w1 = np.asarray(inputs["w1"], dtype=np.float32)
    b1 = np.asarray(inputs["b1"], dtype=np.float32)
    w2 = np.ascontiguousarray(np.asarray(inputs["w2"], dtype=np.float32))
    b2 = np.asarray(inputs["b2"], dtype=np.float32)

    # w1r[p, dc, j] = w1[dc*128 + p, j]
    w1r = np.ascontiguousarray(w1.reshape(DC, P, H1).transpose(1, 0, 2))
    b1r = np.ascontiguousarray(b1.reshape(1, H1))
    b2r = np.ascontiguousarray(b2.reshape(1, H2))

    nb = x.shape[0] // n_cores
    xs = x.reshape(n_cores, nb, S, D)
    in_maps = [
        {
            "x": xs[i],
            "a_vec": a,
            "w1r": w1r,
            "b1r": b1r,
            "w2r": w2,
            "b2r": b2r,
        }
        for i in range(n_cores)
    ]
    return in_maps, nb


def run(inputs, trace=False):
    """Run on hardware; returns (out, w, BassKernelResults)."""
    from concourse.bass_utils import run_bass_kernel_spmd

    in_maps, nb = _prep_maps(inputs)
    nc = _get_nc(nb)
    res = run_bass_kernel_spmd(nc, in_maps, list(range(N_CORES)), trace=trace)
    out = np.concatenate([r["out"] for r in res.results], axis=0)
    w = np.concatenate([r["w_out"] for r in res.results], axis=0)
    return out, w, res


def kernel(**inputs):
    out, w, _ = run(inputs, trace=False)
    return out, w
